# revision 4
# baseline (speedup 1.0000x reference)
"""Trainium2 Bass kernel for MockGCN segment-reduce problem.

Pipeline (per 8-way data-parallel shard, graphs grouped per shard):
  h1 = relu(x @ W_in + b_in)         [N, 64]
  h2 = relu(h1 @ W_h + b_h)          [N, 64]
  pooled[g] = mean_{i in g} h2[i]    [G, 64]
  out = pooled @ W_out + b_out       [G, 5]

Device layout: features-on-partitions ("T orientation"), nodes 2-packed
across the 128 partitions (64 feats x 2 node streams) and 4-interleaved
along the free axis so a column of the partial tensor holds 4 consecutive
nodes.  The host pads every segment to a multiple of 4 nodes, packs
x into xT_dev [128, C4], and the device emits per-4-node-block partial
sums P [128, C4] (bf16).  The host finishes the per-segment combine,
mean division, pad correction, and the tiny [G,64]@[64,5] matmul.

Stage map (per 512-col quantum = 2048 nodes):
  DMA in xT chunk [128,512] f32
  PE:  MM-A (rows 0-63, row-tiled)  -> psum ab[:, 0:512]    (h1raw stream0)
       MM-B (rows 64-127, row-tiled)-> psum ab[:, 512:1024] (h1raw stream1)
  D1:  relu(ab + b1) -> rhs2 [128,1024] f32 SBUF   (ScalarE/VectorE split)
  PE:  MM x2 (K=128, blkdiag(W2,W2)) -> psum cd[:, 0:512], cd[:, 512:1024]
  D2:  relu(cd + b2) -> h2r [128,1024] bf16 SBUF   (ScalarE/VectorE split)
  R:   pc = h2r[:,0:512] + h2r[:,512:1024]  (VectorE TT bf16 2x mode)
  DMA out pc -> partials[:, q*512:(q+1)*512]
"""

import sys

if "/opt/trn_rl_repo" not in sys.path:
    sys.path.insert(0, "/opt/trn_rl_repo")

from contextlib import ExitStack

import ml_dtypes
import numpy as np

N_CORES = 8
G_TOTAL = 8192
F_IN = 32
H_DIM = 64
Q_COLS = 512  # partial columns per quantum
PACK = 4  # nodes per partial column
# Fraction of PSUM-drain ops assigned to ScalarE (rest on VectorE): ACT_NUM/ACT_DEN
ACT_NUM, ACT_DEN = 5, 8

_BUILD_CACHE: dict = {}
_LAST_IN_MAPS: list | None = None


def _build_program(c4: int):
    """Build + compile the 8-core SPMD Bass program for C4 partial columns."""
    import concourse.tile as tile
    from concourse import bacc, mybir

    f32 = mybir.dt.float32
    bf16 = mybir.dt.bfloat16
    Relu = mybir.ActivationFunctionType.Relu
    add_op = mybir.AluOpType.add
    max_op = mybir.AluOpType.max

    nq = c4 // Q_COLS
    assert c4 % Q_COLS == 0

    nc = bacc.Bacc(
        "TRN2",
        target_bir_lowering=False,
        debug=False,
        enable_asserts=False,
        num_devices=N_CORES,
    )

    xT = nc.dram_tensor("xT", [128, c4], f32, kind="ExternalInput").ap()
    w1 = nc.dram_tensor("w1", [128, 128], f32, kind="ExternalInput").ap()
    w2 = nc.dram_tensor("w2", [128, 128], f32, kind="ExternalInput").ap()
    b1 = nc.dram_tensor("b1", [128, 1], f32, kind="ExternalInput").ap()
    b2 = nc.dram_tensor("b2", [128, 1], f32, kind="ExternalInput").ap()
    pout = nc.dram_tensor("pout", [128, c4], bf16, kind="ExternalOutput").ap()

    # Bresenham schedule for the ScalarE/VectorE drain split.
    def drain_on_act(k: int) -> bool:
        return ((k + 1) * ACT_NUM) // ACT_DEN > (k * ACT_NUM) // ACT_DEN

    with ExitStack() as ctx:
        tc = ctx.enter_context(tile.TileContext(nc))
        singles = ctx.enter_context(tc.tile_pool(name="singles", bufs=1))
        xpool = ctx.enter_context(tc.tile_pool(name="xc", bufs=4))
        rpool = ctx.enter_context(tc.tile_pool(name="rhs2", bufs=3))
        hpool = ctx.enter_context(tc.tile_pool(name="h2r", bufs=3))
        ppool = ctx.enter_context(tc.tile_pool(name="pc", bufs=4))
        abpool = ctx.enter_context(tc.tile_pool(name="ab", bufs=2, space="PSUM"))
        cdpool = ctx.enter_context(tc.tile_pool(name="cd", bufs=2, space="PSUM"))

        w1sb = singles.tile([128, 128], f32)
        w2sb = singles.tile([128, 128], f32)
        b1sb = singles.tile([128, 1], f32)
        b2sb = singles.tile([128, 1], f32)
        nc.sync.dma_start(out=w1sb, in_=w1)
        nc.sync.dma_start(out=w2sb, in_=w2)
        nc.sync.dma_start(out=b1sb, in_=b1)
        nc.sync.dma_start(out=b2sb, in_=b2)

        dk = 0  # drain-op index for the engine split schedule
        for q in range(nq):
            cols = slice(q * Q_COLS, (q + 1) * Q_COLS)

            xc = xpool.tile([128, Q_COLS], f32)
            nc.sync.dma_start(out=xc, in_=xT[:, cols])

            ab = abpool.tile([128, 2 * Q_COLS], f32)
            # Stage 1: two row-tiled matmuls run concurrently on PE.
            nc.tensor.matmul(
                out=ab[:, 0:Q_COLS],
                lhsT=w1sb[0:64, :],
                rhs=xc[0:64, :],
                start=True,
                stop=True,
            )
            nc.tensor.matmul(
                out=ab[:, Q_COLS : 2 * Q_COLS],
                lhsT=w1sb[64:128, :],
                rhs=xc[64:128, :],
                start=True,
                stop=True,
            )

            rhs2 = rpool.tile([128, 2 * Q_COLS], f32)
            if drain_on_act(dk):
                nc.scalar.activation(rhs2, ab, Relu, bias=b1sb)
            else:
                nc.vector.tensor_scalar(rhs2, ab, b1sb, 0.0, add_op, max_op)
            dk += 1

            cd = cdpool.tile([128, 2 * Q_COLS], f32)
            nc.tensor.matmul(
                out=cd[:, 0:Q_COLS],
                lhsT=w2sb,
                rhs=rhs2[:, 0:Q_COLS],
                start=True,
                stop=True,
            )
            nc.tensor.matmul(
                out=cd[:, Q_COLS : 2 * Q_COLS],
                lhsT=w2sb,
                rhs=rhs2[:, Q_COLS : 2 * Q_COLS],
                start=True,
                stop=True,
            )

            h2r = hpool.tile([128, 2 * Q_COLS], bf16)
            if drain_on_act(dk):
                nc.scalar.activation(h2r, cd, Relu, bias=b2sb)
            else:
                nc.vector.tensor_scalar(h2r, cd, b2sb, 0.0, add_op, max_op)
            dk += 1

            pc = ppool.tile([128, Q_COLS], bf16)
            nc.vector.tensor_add(pc, h2r[:, 0:Q_COLS], h2r[:, Q_COLS : 2 * Q_COLS])
            nc.sync.dma_start(out=pout[:, cols], in_=pc)

    nc.compile()
    return nc


def _get_program(c4: int):
    if c4 not in _BUILD_CACHE:
        _BUILD_CACHE[c4] = _build_program(c4)
    return _BUILD_CACHE[c4]


def kernel(x, batch, num_graphs, W_in, b_in, W_h, b_h, W_out, b_out):
    from concourse import bass_utils

    x = np.asarray(x, dtype=np.float32)
    batch = np.asarray(batch).astype(np.int64)
    g_total = int(num_graphs)
    W_in = np.asarray(W_in, dtype=np.float32)
    b_in = np.asarray(b_in, dtype=np.float32)
    W_h = np.asarray(W_h, dtype=np.float32)
    b_h = np.asarray(b_h, dtype=np.float32)
    W_out = np.asarray(W_out, dtype=np.float32)
    b_out = np.asarray(b_out, dtype=np.float32)

    n_nodes, f_in = x.shape
    h_dim = W_in.shape[1]
    assert f_in == F_IN and h_dim == H_DIM
    assert g_total % N_CORES == 0
    g_per_core = g_total // N_CORES

    counts = np.bincount(batch, minlength=g_total).astype(np.int64)
    node_starts = np.concatenate([[0], np.cumsum(counts)])  # [G+1]

    # Per-graph padded counts (multiple of PACK).
    pc_counts = (counts + PACK - 1) // PACK * PACK

    # Per-core geometry.
    core_g0 = [c * g_per_core for c in range(N_CORES)]
    core_pad_tot = [
        int(pc_counts[c * g_per_core : (c + 1) * g_per_core].sum())
        for c in range(N_CORES)
    ]
    c4_per_core = [t // PACK for t in core_pad_tot]
    c4 = max(c4_per_core)
    c4 = (c4 + Q_COLS - 1) // Q_COLS * Q_COLS  # uniform, quantum-aligned

    # Constant tensors shared by all cores.
    w1blk = np.zeros((128, 128), dtype=np.float32)
    w1blk[0:32, 0:64] = W_in
    w1blk[32:64, 64:128] = W_in
    w1blk[64:96, 0:64] = W_in
    w1blk[96:128, 64:128] = W_in
    w2blk = np.zeros((128, 128), dtype=np.float32)
    w2blk[0:64, 0:64] = W_h
    w2blk[64:128, 64:128] = W_h
    b1cat = np.tile(b_in, 2).reshape(128, 1).astype(np.float32)
    b2cat = np.tile(b_h, 2).reshape(128, 1).astype(np.float32)

    # Per-core packed inputs.
    in_maps = []
    for c in range(N_CORES):
        g0 = core_g0[c]
        g1 = g0 + g_per_core
        s, e = int(node_starts[g0]), int(node_starts[g1])
        cnt_c = counts[g0:g1]
        pc_c = pc_counts[g0:g1]
        pad_starts = np.concatenate([[0], np.cumsum(pc_c)])  # [g_per_core+1]

        x_padded = np.zeros((c4 * PACK, f_in), dtype=np.float32)
        if e > s:
            local_batch = batch[s:e] - g0
            # dst = pad_start of graph + index within graph
            dst = pad_starts[local_batch] + (
                np.arange(s, e) - node_starts[g0 + local_batch]
            )
            x_padded[dst] = x[s:e]
        xT_dev = (
            x_padded.reshape(c4, PACK, f_in).transpose(1, 2, 0).reshape(128, c4)
        )
        xT_dev = np.ascontiguousarray(xT_dev, dtype=np.float32)
        in_maps.append(
            {
                "xT": xT_dev,
                "w1": w1blk,
                "w2": w2blk,
                "b1": b1cat,
                "b2": b2cat,
            }
        )

    global _LAST_IN_MAPS
    _LAST_IN_MAPS = in_maps

    nc = _get_program(c4)
    res = bass_utils.run_bass_kernel_spmd(
        nc, in_maps, core_ids=list(range(N_CORES))
    )

    # Pad-node contribution, exactly as the device computes it for x=0 rows.
    vpad = np.maximum(np.maximum(b_in, 0.0) @ W_h + b_h, 0.0)
    vpad_bf = vpad.astype(ml_dtypes.bfloat16).astype(np.float32)

    out = np.zeros((g_total, W_out.shape[1]), dtype=np.float32)
    for c in range(N_CORES):
        g0 = core_g0[c]
        g1 = g0 + g_per_core
        cnt_c = counts[g0:g1].astype(np.float64)
        pc_c = pc_counts[g0:g1]
        pad_starts = np.concatenate([[0], np.cumsum(pc_c)])
        col_starts = pad_starts // PACK  # [g_per_core+1]

        P = np.asarray(res.results[c]["pout"]).astype(np.float32)  # [128, c4]
        R1 = P[0:64, :] + P[64:128, :]  # [64, c4]
        cs = np.concatenate(
            [np.zeros((64, 1)), np.cumsum(R1.astype(np.float64), axis=1)], axis=1
        )  # [64, c4+1]
        seg_sum = (cs[:, col_starts[1:]] - cs[:, col_starts[:-1]]).T  # [g, 64]

        n_pad = (pc_c - counts[g0:g1]).astype(np.float64)
        seg_sum = seg_sum - n_pad[:, None] * vpad_bf[None, :].astype(np.float64)
        denom = np.maximum(cnt_c, 1.0)
        mean = seg_sum / denom[:, None]
        mean[cnt_c == 0] = 0.0
        out[g0:g1] = mean.astype(np.float32) @ W_out + b_out

    return out
